# revision 38
# baseline (speedup 1.0000x reference)
"""Trainium2 Bass kernel for single-head causal attention.

  q = Xq @ Wq.T + bq ; k = Xk @ Wk.T + bk ; v = Xv @ Wv.T + bv
  out = softmax((q k^T + causal_mask)/sqrt(D)) @ v

Shapes: B=4, S=2048, D=1024, fp32 in/out.  8 NeuronCores, SPMD.

Sharding: core c handles batch b = c//2, parity h = c%2.  S splits into 16
q-tiles of 128; causal attention for q-tile g touches k-tiles 0..g.  Core
parity h owns q-tiles g = 2j + h (j = 0..7), and slot j statically
processes 2j+2 k-tiles on BOTH parities (identical SPMD program); the
h=0 core's last k-tile per slot is fully masked, so the per-core causal
mask is one static [128, 256] tile covering the last two k-tiles.

Compute (all bf16 matmuls, f32 psum):
  - K^T, Q^T projected to [e-part, s] layout, V to [s-part, d]; all three
    stay fully SBUF-resident (no DRAM scratch roundtrip).
  - Scores are computed TRANSPOSED ([k, q] blocks): exp output feeds the
    P@V matmul directly as the stationary operand - no PE transposes.
  - softmax denominator = pe-block matmul against a ones column, giving
    [q-part, 1] psum, the right orientation for the final normalize
    (out = av * (1/den) + bv on DVE).
  - attention is software-pipelined one slot deep: scores(j+1) are issued
    before P@V(j) so the exp never stalls the tensor engine.
"""

from contextlib import ExitStack

import ml_dtypes
import numpy as np

import concourse.bacc as bacc
import concourse.mybir as mybir
import concourse.tile as tile
from concourse.bass_utils import run_bass_kernel_spmd

P = 128
D = 1024
S = 2048
B = 4
N_CORES = 8
EO = D // P            # 8 contraction chunks of 128
DO = D // P            # 8 output-dim chunks of 128
NT = S // P            # 16 k/s tiles of 128
NQ = 8                 # q-tile slots per core
F32 = mybir.dt.float32
BF16 = mybir.dt.bfloat16
FP8 = mybir.dt.float8e4
NEG = -1.0e9
BF = ml_dtypes.bfloat16

_PROG_CACHE = {}


def _slot_gtiles(h, causal):
    """q-tile ids (units of 128 rows) owned by parity-h core, slot order."""
    if causal:
        return [2 * j + h for j in range(NQ)]
    return [8 * h + j for j in range(NQ)]


def build_program(causal: bool):
    nc = bacc.Bacc(trn_type="TRN2", target_bir_lowering=False, debug=False)

    def din(name, shape, dt=BF16):
        return nc.dram_tensor(name, shape, dt, kind="ExternalInput").ap()

    xq = din("xq", [P, EO, 1024], FP8)   # Xq^T for this core's 8 q-tiles
    xk = din("xk", [4, P, EO, 512], FP8)  # Xk^T, chunked along s
    xv = din("xv", [NT, P, EO, P])       # Xv^T, blocked [s-tile][e][s]
    wq = din("wq", [P, EO, D], FP8)
    wk = din("wk", [P, EO, D], FP8)
    wv = din("wv", [P, EO, D])
    bq = din("bq", [P, DO], F32)
    bk = din("bk", [P, DO], F32)
    bv = din("bv", [P, D], F32)
    msk = din("msk", [P, 2 * P], F32)    # causal mask for last 2 k-tiles
    out = nc.dram_tensor("out", [NQ, P, D], F32, kind="ExternalOutput").ap()

    Ident = mybir.ActivationFunctionType.Identity
    Copy = mybir.ActivationFunctionType.Copy
    Exp = mybir.ActivationFunctionType.Exp
    add = mybir.AluOpType.add
    mult = mybir.AluOpType.mult

    # slot j processes nkt[j] k-tiles - identical on every core
    nkt = [2 * j + 2 if causal else NT for j in range(NQ)]

    with tile.TileContext(nc, pool_alloc_mode="queue") as tc, ExitStack() as top:
        const = top.enter_context(tc.tile_pool(name="const", bufs=1))
        bq_sb = const.tile([P, DO], F32)
        bk_sb = const.tile([P, DO], F32)
        bv_sb = const.tile([P, D], F32)
        msk_sb = const.tile([P, 2 * P], F32)
        ones_sb = const.tile([P, 1], BF16)
        nc.gpsimd.memset(ones_sb, 1.0)

        # resident projected tensors
        res = top.enter_context(tc.tile_pool(name="res", bufs=1))
        kt_sb = res.tile([P, DO, S], FP8, name="kt_sb")      # K^T [e, k]
        qt_sb = res.tile([P, DO, 1024], FP8, name="qt_sb")   # Q^T [e, q]
        v_sb = res.tile([P, NT, D], BF16, name="v_sb")       # V [s, d] blocked

        # ---------------- projections ----------------
        with tc.tile_pool(name="wt", bufs=2) as wtp, \
             tc.tile_pool(name="xin", bufs=4) as xinp, \
             tc.tile_pool(name="xqp", bufs=1) as xqp, \
             tc.tile_pool(name="xvp", bufs=6) as xvp, \
             tc.tile_pool(name="psA", bufs=3, space="PSUM") as psA, \
             tc.tile_pool(name="psB", bufs=2, space="PSUM") as psB:

            # V projection FIRST: bf16 (1 cyc/row) gives the fp8 K/Q
            # inputs the whole phase to land, so the (2x faster, DMA-hungry)
            # DoubleRow projections never starve.
            qs = [nc.sync, nc.scalar, nc.gpsimd]
            wv_sb = wtp.tile([P, EO, D], BF16, tag="wt", name="wv_sb")
            xv_t0 = xvp.tile([P, EO, P], BF16, tag="xv", name="xv_t0")
            # half-0 pieces first: the V matmul loop is half-major, so st0's
            # first psum group needs only wv[:, :, 0:512] + xv0 to complete
            for half in range(2):
                for eo in range(EO):
                    i = half * EO + eo
                    qs[i % 3].dma_start(
                        out=wv_sb[:, eo, half * 512:(half + 1) * 512],
                        in_=wv[:, eo, half * 512:(half + 1) * 512])
                    if half == 0:
                        qs[(i + 1) % 3].dma_start(out=xv_t0[:, eo, :],
                                                  in_=xv[0, :, eo, :])
            # consts are needed only at the first psum eviction; issue them
            # after the startup-critical wave
            nc.gpsimd.dma_start(out=bk_sb, in_=bk)
            nc.gpsimd.dma_start(out=bq_sb, in_=bq)
            nc.gpsimd.dma_start(out=msk_sb, in_=msk)
            nc.gpsimd.dma_start(out=bv_sb, in_=bv)
            # K/Q inputs: single big transfers, they have the V phase to land
            wk_sb = wtp.tile([P, EO, D], FP8, tag="wt", name="wk_sb")
            nc.scalar.dma_start(out=wk_sb, in_=wk)
            xk_ts = []
            for kc in range(4):
                xk_t = xinp.tile([P, EO, 512], FP8, tag="xin", name=f"xk_t{kc}")
                xk_ts.append(xk_t)
                nc.scalar.dma_start(out=xk_t, in_=xk[kc])
            wq_sb = wtp.tile([P, EO, D], FP8, tag="wt", name="wq_sb")
            xq_t = xqp.tile([P, EO, 1024], FP8, name="xq_t")
            nc.scalar.dma_start(out=wq_sb, in_=wq)
            nc.scalar.dma_start(out=xq_t, in_=xq)

            for st in range(NT):
                if st == 0:
                    xv_t = xv_t0
                else:
                    xv_t = xvp.tile([P, EO, P], BF16, tag="xv", name=f"xv_t{st}")
                    if st <= 3:
                        # startup-critical: piece-split across both queues so
                        # the first tiles arrive in parallel
                        for e2 in range(0, EO, 2):
                            eng = nc.sync if (st + e2 // 2) % 2 == 0 else nc.gpsimd
                            eng.dma_start(out=xv_t[:, e2:e2 + 2, :],
                                          in_=xv[st, :, e2:e2 + 2, :])
                    else:
                        eng = nc.sync if st % 2 == 0 else nc.gpsimd
                        eng.dma_start(out=xv_t, in_=xv[st])
                ps2 = psB.tile([P, D], F32, tag="psB", name=f"psv{st}")
                for half in range(2):
                    for eo in range(EO):
                        nc.tensor.matmul(
                            ps2[:, half * 512:(half + 1) * 512],
                            lhsT=xv_t[:, eo, :],
                            rhs=wv_sb[:, eo, half * 512:(half + 1) * 512],
                            start=(eo == 0), stop=(eo == EO - 1))
                # evict on Act so DVE stays free for attention-phase work
                nc.scalar.activation(out=v_sb[:, st, :], in_=ps2, func=Copy)

            # K projection -> kt_sb [e-part, k], bias folded in (fp8 DoubleRow)
            for kc in range(4):
                xk_t = xk_ts[kc]
                for do in range(DO):
                    ps = psA.tile([P, 512], F32, tag="psA", name=f"psk{kc}_{do}")
                    for m in range(4):
                        nc.tensor.matmul(
                            ps,
                            lhsT=wk_sb[:, 2 * m:2 * m + 2, do * P:(do + 1) * P],
                            rhs=xk_t[:, 2 * m:2 * m + 2, :],
                            start=(m == 0), stop=(m == 3),
                            perf_mode=mybir.MatmulPerfMode.DoubleRow)
                    nc.scalar.activation(
                        out=kt_sb[:, do, kc * 512:(kc + 1) * 512], in_=ps,
                        func=Ident, bias=bk_sb[:, do:do + 1], scale=1.0 / 16)

            # Q projection -> qt_sb [e-part, q], bias folded in (fp8 DoubleRow)
            for sc in range(2):
                for do in range(DO):
                    ps = psA.tile([P, 512], F32, tag="psA", name=f"psq{sc}_{do}")
                    for m in range(4):
                        nc.tensor.matmul(
                            ps,
                            lhsT=wq_sb[:, 2 * m:2 * m + 2, do * P:(do + 1) * P],
                            rhs=xq_t[:, 2 * m:2 * m + 2, sc * 512:(sc + 1) * 512],
                            start=(m == 0), stop=(m == 3),
                            perf_mode=mybir.MatmulPerfMode.DoubleRow)
                    nc.scalar.activation(
                        out=qt_sb[:, do, sc * 512:(sc + 1) * 512], in_=ps,
                        func=Ident, bias=bq_sb[:, do:do + 1], scale=1.0 / 16)

        # ---------------- attention ----------------
        # scores k-chunk-major: one stationary K-tile streams against ALL
        # active slots' Q columns (slots active for chunk c are j >= 2c,
        # contiguous in qt) - amortizes the fp8 weight loads.  exp lands in
        # pe_all[k-tile][slot*128], then P@V runs slot-major as before.
        with tc.tile_pool(name="pep", bufs=1) as pep, \
             tc.tile_pool(name="recp", bufs=2) as recp, \
             tc.tile_pool(name="outp", bufs=2) as outp, \
             tc.tile_pool(name="psS", bufs=3, space="PSUM") as psS, \
             tc.tile_pool(name="psV", bufs=2, space="PSUM") as psV, \
             tc.tile_pool(name="psD", bufs=1, space="PSUM") as psD:

            pe_all = pep.tile([P, NT, 1024], BF16, name="pe_all")
            dn_ps = psD.tile([P, NQ], F32, tag="dn", name="dn_ps")

            for c in range(4):
                j0 = 2 * c if causal else 0
                w = (NQ - j0) * P
                for i in range(4):
                    t = 4 * c + i
                    dcol = (t // 2 - j0) * P        # diagonal slot's column
                    for p0 in range(0, w, 512):
                        wp = min(512, w - p0)
                        ps = psS.tile([P, wp], F32, tag="s", name=f"ps{t}_{p0}")
                        for m in range(4):
                            nc.tensor.matmul(
                                ps,
                                lhsT=kt_sb[:, 2 * m:2 * m + 2, t * P:(t + 1) * P],
                                rhs=qt_sb[:, 2 * m:2 * m + 2,
                                          j0 * P + p0:j0 * P + p0 + wp],
                                start=(m == 0), stop=(m == 3),
                                perf_mode=mybir.MatmulPerfMode.DoubleRow)
                        if causal and p0 <= dcol < p0 + wp:
                            nc.vector.tensor_tensor(
                                out=ps[:, dcol - p0:dcol - p0 + P],
                                in0=ps[:, dcol - p0:dcol - p0 + P],
                                in1=msk_sb[:, (t % 2) * P:(t % 2 + 1) * P],
                                op=add)
                        nc.scalar.activation(
                            out=pe_all[:, t, j0 * P + p0:j0 * P + p0 + wp],
                            in_=ps, func=Exp, scale=float(1.0 / np.sqrt(D)))

            def av_slot(j):
                n_t = nkt[j]
                av = psV.tile([P, D], F32, tag="av", name=f"av{j}")
                for t in range(n_t):
                    pblk = pe_all[:, t, j * P:(j + 1) * P]
                    nc.tensor.matmul(
                        dn_ps[:, j:j + 1], lhsT=pblk, rhs=ones_sb,
                        start=(t == 0), stop=(t == n_t - 1))
                    for half in range(2):
                        nc.tensor.matmul(
                            av[:, half * 512:(half + 1) * 512],
                            lhsT=pblk,
                            rhs=v_sb[:, t, half * 512:(half + 1) * 512],
                            start=(t == 0), stop=(t == n_t - 1))
                rec = recp.tile([P, 1], F32, tag="rec", name=f"rec{j}")
                nc.vector.reciprocal(out=rec, in_=dn_ps[:, j:j + 1])
                o = outp.tile([P, D], F32, tag="o", name=f"o{j}")
                last = j == (0 if causal else NQ - 1)
                nhf = 4 if last else 2
                hw = D // nhf
                for hf in range(nhf):
                    nc.vector.scalar_tensor_tensor(
                        out=o[:, hf * hw:(hf + 1) * hw],
                        in0=av[:, hf * hw:(hf + 1) * hw], scalar=rec,
                        in1=bv_sb[:, hf * hw:(hf + 1) * hw],
                        op0=mult, op1=add)
                    eng = nc.sync if hf % 2 == 0 else nc.scalar
                    eng.dma_start(out=out[j, :, hf * hw:(hf + 1) * hw],
                                  in_=o[:, hf * hw:(hf + 1) * hw])

            # ascending so early AVs only need early exps; smallest slot
            # last so the kernel tail is its short drain
            for j in ([1, 2, 3, 4, 5, 6, 7, 0] if causal else list(range(NQ))):
                av_slot(j)

    nc.compile()
    return nc


def _get_program(causal: bool):
    key = bool(causal)
    if key not in _PROG_CACHE:
        _PROG_CACHE[key] = build_program(key)
    return _PROG_CACHE[key]


def _shard_inputs(encoded_q, encoded_k, encoded_v, W_q, b_q, W_k, b_k,
                  W_v, b_v, causal):
    """Build the per-core in_maps (all host-side numpy, bf16 payloads)."""
    F8 = mybir.dt.np(FP8)
    wqh = np.ascontiguousarray(
        (16.0 * W_q.T).reshape(EO, P, D).transpose(1, 0, 2)).astype(F8)
    wkh = np.ascontiguousarray(
        (16.0 * W_k.T).reshape(EO, P, D).transpose(1, 0, 2)).astype(F8)
    wvh = np.ascontiguousarray(
        W_v.T.reshape(EO, P, D).transpose(1, 0, 2)).astype(BF)
    bqh = np.ascontiguousarray(b_q.reshape(DO, P).T)
    bkh = np.ascontiguousarray(b_k.reshape(DO, P).T)
    bvh = np.ascontiguousarray(np.broadcast_to(b_v, (P, D)))

    ki = np.arange(P)[:, None]
    qi = np.arange(P)[None, :]
    tri = np.where(ki <= qi, 0.0, NEG).astype(np.float32)   # diagonal block
    zer = np.zeros((P, P), np.float32)
    ninf = np.full((P, P), NEG, np.float32)
    # h=0: slot j owns g=2j -> k-tile 2j is diagonal, 2j+1 fully masked
    # h=1: slot j owns g=2j+1 -> k-tile 2j unmasked, 2j+1 diagonal
    mskh = [np.concatenate([tri, ninf], 1), np.concatenate([zer, tri], 1)]

    in_maps = []
    for c in range(N_CORES):
        b, h = divmod(c, 2)
        gts = _slot_gtiles(h, causal)
        Xq = np.concatenate([encoded_q[b, g * P:(g + 1) * P, :] for g in gts], 0)
        xqh = np.ascontiguousarray(
            Xq.T.reshape(EO, P, 1024).transpose(1, 0, 2)).astype(F8)
        xkh = np.ascontiguousarray(
            encoded_k[b].T.reshape(EO, P, 4, 512).transpose(2, 1, 0, 3)).astype(F8)
        xvh = np.ascontiguousarray(
            encoded_v[b].T.reshape(EO, P, NT, P).transpose(2, 1, 0, 3)).astype(BF)
        in_maps.append({
            "xq": xqh, "xk": xkh, "xv": xvh,
            "wq": wqh, "wk": wkh, "wv": wvh,
            "bq": bqh, "bk": bkh, "bv": bvh,
            "msk": mskh[h] if causal else np.zeros((P, 2 * P), np.float32),
        })
    return in_maps


def kernel(encoded_q, encoded_k, encoded_v, W_q, b_q, W_k, b_k, W_v, b_v,
           parameter_mask, _want_trace=False, _trace_dir=None):
    causal = bool(np.asarray(parameter_mask).item())
    encoded_q = np.asarray(encoded_q, np.float32)
    encoded_k = np.asarray(encoded_k, np.float32)
    encoded_v = np.asarray(encoded_v, np.float32)
    nc = _get_program(causal)
    in_maps = _shard_inputs(encoded_q, encoded_k, encoded_v,
                            np.asarray(W_q, np.float32), np.asarray(b_q, np.float32),
                            np.asarray(W_k, np.float32), np.asarray(b_k, np.float32),
                            np.asarray(W_v, np.float32), np.asarray(b_v, np.float32),
                            causal)
    kw = {}
    if _want_trace:
        kw = dict(trace=True, tmpdir=_trace_dir)
    res = run_bass_kernel_spmd(nc, in_maps, core_ids=list(range(N_CORES)), **kw)

    full = np.empty((B, S, D), np.float32)
    for c in range(N_CORES):
        b, h = divmod(c, 2)
        o = res.results[c]["out"]
        for j, g in enumerate(_slot_gtiles(h, causal)):
            full[b, g * P:(g + 1) * P, :] = o[j]
    if _want_trace:
        return full, res
    return full


# revision 39
# speedup vs baseline: 1.0091x; 1.0091x over previous
"""Trainium2 Bass kernel for single-head causal attention.

  q = Xq @ Wq.T + bq ; k = Xk @ Wk.T + bk ; v = Xv @ Wv.T + bv
  out = softmax((q k^T + causal_mask)/sqrt(D)) @ v

Shapes: B=4, S=2048, D=1024, fp32 in/out.  8 NeuronCores, SPMD.

Sharding: core c handles batch b = c//2, parity h = c%2.  S splits into 16
q-tiles of 128; causal attention for q-tile g touches k-tiles 0..g.  Core
parity h owns q-tiles g = 2j + h (j = 0..7), and slot j statically
processes 2j+2 k-tiles on BOTH parities (identical SPMD program); the
h=0 core's last k-tile per slot is fully masked, so the per-core causal
mask is one static [128, 256] tile covering the last two k-tiles.

Compute (all bf16 matmuls, f32 psum):
  - K^T, Q^T projected to [e-part, s] layout, V to [s-part, d]; all three
    stay fully SBUF-resident (no DRAM scratch roundtrip).
  - Scores are computed TRANSPOSED ([k, q] blocks): exp output feeds the
    P@V matmul directly as the stationary operand - no PE transposes.
  - softmax denominator = pe-block matmul against a ones column, giving
    [q-part, 1] psum, the right orientation for the final normalize
    (out = av * (1/den) + bv on DVE).
  - attention is software-pipelined one slot deep: scores(j+1) are issued
    before P@V(j) so the exp never stalls the tensor engine.
"""

from contextlib import ExitStack

import ml_dtypes
import numpy as np

import concourse.bacc as bacc
import concourse.mybir as mybir
import concourse.tile as tile
from concourse.bass_utils import run_bass_kernel_spmd

P = 128
D = 1024
S = 2048
B = 4
N_CORES = 8
EO = D // P            # 8 contraction chunks of 128
DO = D // P            # 8 output-dim chunks of 128
NT = S // P            # 16 k/s tiles of 128
NQ = 8                 # q-tile slots per core
F32 = mybir.dt.float32
BF16 = mybir.dt.bfloat16
FP8 = mybir.dt.float8e4
NEG = -1.0e9
BF = ml_dtypes.bfloat16

_PROG_CACHE = {}


def _slot_gtiles(h, causal):
    """q-tile ids (units of 128 rows) owned by parity-h core, slot order."""
    if causal:
        return [2 * j + h for j in range(NQ)]
    return [8 * h + j for j in range(NQ)]


def build_program(causal: bool):
    nc = bacc.Bacc(trn_type="TRN2", target_bir_lowering=False, debug=False)

    def din(name, shape, dt=BF16):
        return nc.dram_tensor(name, shape, dt, kind="ExternalInput").ap()

    xq = din("xq", [P, EO, 1024], FP8)   # Xq^T for this core's 8 q-tiles
    xk = din("xk", [4, P, EO, 512], FP8)  # Xk^T, chunked along s
    xv = din("xv", [NT, P, EO, P])       # Xv^T, blocked [s-tile][e][s]
    wq = din("wq", [P, EO, D], FP8)
    wk = din("wk", [P, EO, D], FP8)
    wv = din("wv", [P, EO, D])
    bq = din("bq", [P, DO], F32)
    bk = din("bk", [P, DO], F32)
    bv = din("bv", [P, D], F32)
    msk = din("msk", [P, 2 * P], F32)    # causal mask for last 2 k-tiles
    out = nc.dram_tensor("out", [NQ, P, D], F32, kind="ExternalOutput").ap()

    Ident = mybir.ActivationFunctionType.Identity
    Copy = mybir.ActivationFunctionType.Copy
    Exp = mybir.ActivationFunctionType.Exp
    add = mybir.AluOpType.add
    mult = mybir.AluOpType.mult

    # slot j processes nkt[j] k-tiles - identical on every core
    nkt = [2 * j + 2 if causal else NT for j in range(NQ)]

    with tile.TileContext(nc, pool_alloc_mode="queue") as tc, ExitStack() as top:
        const = top.enter_context(tc.tile_pool(name="const", bufs=1))
        bq_sb = const.tile([P, DO], F32)
        bk_sb = const.tile([P, DO], F32)
        bv_sb = const.tile([P, D], F32)
        msk_sb = const.tile([P, 2 * P], F32)
        ones_sb = const.tile([P, 1], BF16)
        nc.gpsimd.memset(ones_sb, 1.0)

        # resident projected tensors
        res = top.enter_context(tc.tile_pool(name="res", bufs=1))
        kt_sb = res.tile([P, DO, S], FP8, name="kt_sb")      # K^T [e, k]
        qt_sb = res.tile([P, DO, 1024], FP8, name="qt_sb")   # Q^T [e, q]
        v_sb = res.tile([P, NT, D], BF16, name="v_sb")       # V [s, d] blocked

        # ---------------- projections ----------------
        with tc.tile_pool(name="wt", bufs=2) as wtp, \
             tc.tile_pool(name="xin", bufs=4) as xinp, \
             tc.tile_pool(name="xqp", bufs=1) as xqp, \
             tc.tile_pool(name="xvp", bufs=6) as xvp, \
             tc.tile_pool(name="psA", bufs=3, space="PSUM") as psA, \
             tc.tile_pool(name="psB", bufs=2, space="PSUM") as psB:

            # V projection FIRST: bf16 (1 cyc/row) gives the fp8 K/Q
            # inputs the whole phase to land, so the (2x faster, DMA-hungry)
            # DoubleRow projections never starve.
            qs = [nc.sync, nc.scalar, nc.gpsimd]
            wv_sb = wtp.tile([P, EO, D], BF16, tag="wt", name="wv_sb")
            xv_t0 = xvp.tile([P, EO, P], BF16, tag="xv", name="xv_t0")
            for eo in range(EO):
                qs[eo % 3].dma_start(out=wv_sb[:, eo, :], in_=wv[:, eo, :])
                qs[(eo + 1) % 3].dma_start(out=xv_t0[:, eo, :], in_=xv[0, :, eo, :])
            # consts are needed only at the first psum eviction; issue them
            # after the startup-critical wave
            nc.gpsimd.dma_start(out=bk_sb, in_=bk)
            nc.gpsimd.dma_start(out=bq_sb, in_=bq)
            nc.gpsimd.dma_start(out=msk_sb, in_=msk)
            nc.gpsimd.dma_start(out=bv_sb, in_=bv)
            # K/Q inputs: single big transfers, they have the V phase to land
            wk_sb = wtp.tile([P, EO, D], FP8, tag="wt", name="wk_sb")
            nc.scalar.dma_start(out=wk_sb, in_=wk)
            xk_ts = []
            for kc in range(4):
                xk_t = xinp.tile([P, EO, 512], FP8, tag="xin", name=f"xk_t{kc}")
                xk_ts.append(xk_t)
                nc.scalar.dma_start(out=xk_t, in_=xk[kc])
            wq_sb = wtp.tile([P, EO, D], FP8, tag="wt", name="wq_sb")
            xq_t = xqp.tile([P, EO, 1024], FP8, name="xq_t")
            nc.scalar.dma_start(out=wq_sb, in_=wq)
            nc.scalar.dma_start(out=xq_t, in_=xq)

            for st in range(NT):
                if st == 0:
                    xv_t = xv_t0
                else:
                    xv_t = xvp.tile([P, EO, P], BF16, tag="xv", name=f"xv_t{st}")
                    if st <= 3:
                        # startup-critical: piece-split across both queues so
                        # the first tiles arrive in parallel
                        for e2 in range(0, EO, 2):
                            eng = nc.sync if (st + e2 // 2) % 2 == 0 else nc.gpsimd
                            eng.dma_start(out=xv_t[:, e2:e2 + 2, :],
                                          in_=xv[st, :, e2:e2 + 2, :])
                    else:
                        eng = nc.sync if st % 2 == 0 else nc.gpsimd
                        eng.dma_start(out=xv_t, in_=xv[st])
                ps2 = psB.tile([P, D], F32, tag="psB", name=f"psv{st}")
                for half in range(2):
                    for eo in range(EO):
                        nc.tensor.matmul(
                            ps2[:, half * 512:(half + 1) * 512],
                            lhsT=xv_t[:, eo, :],
                            rhs=wv_sb[:, eo, half * 512:(half + 1) * 512],
                            start=(eo == 0), stop=(eo == EO - 1))
                # evict on Act so DVE stays free for attention-phase work
                nc.scalar.activation(out=v_sb[:, st, :], in_=ps2, func=Copy)

            # K projection -> kt_sb [e-part, k], bias folded in (fp8 DoubleRow)
            for kc in range(4):
                xk_t = xk_ts[kc]
                for do in range(DO):
                    ps = psA.tile([P, 512], F32, tag="psA", name=f"psk{kc}_{do}")
                    for m in range(4):
                        nc.tensor.matmul(
                            ps,
                            lhsT=wk_sb[:, 2 * m:2 * m + 2, do * P:(do + 1) * P],
                            rhs=xk_t[:, 2 * m:2 * m + 2, :],
                            start=(m == 0), stop=(m == 3),
                            perf_mode=mybir.MatmulPerfMode.DoubleRow)
                    nc.scalar.activation(
                        out=kt_sb[:, do, kc * 512:(kc + 1) * 512], in_=ps,
                        func=Ident, bias=bk_sb[:, do:do + 1], scale=1.0 / 16)

            # Q projection -> qt_sb [e-part, q], bias folded in (fp8 DoubleRow)
            for sc in range(2):
                for do in range(DO):
                    ps = psA.tile([P, 512], F32, tag="psA", name=f"psq{sc}_{do}")
                    for m in range(4):
                        nc.tensor.matmul(
                            ps,
                            lhsT=wq_sb[:, 2 * m:2 * m + 2, do * P:(do + 1) * P],
                            rhs=xq_t[:, 2 * m:2 * m + 2, sc * 512:(sc + 1) * 512],
                            start=(m == 0), stop=(m == 3),
                            perf_mode=mybir.MatmulPerfMode.DoubleRow)
                    nc.scalar.activation(
                        out=qt_sb[:, do, sc * 512:(sc + 1) * 512], in_=ps,
                        func=Ident, bias=bq_sb[:, do:do + 1], scale=1.0 / 16)

        # ---------------- attention ----------------
        # scores k-chunk-major: one stationary K-tile streams against ALL
        # active slots' Q columns (slots active for chunk c are j >= 2c,
        # contiguous in qt) - amortizes the fp8 weight loads.  exp lands in
        # pe_all[k-tile][slot*128], then P@V runs slot-major as before.
        with tc.tile_pool(name="pep", bufs=1) as pep, \
             tc.tile_pool(name="recp", bufs=2) as recp, \
             tc.tile_pool(name="outp", bufs=2) as outp, \
             tc.tile_pool(name="psS", bufs=3, space="PSUM") as psS, \
             tc.tile_pool(name="psV", bufs=2, space="PSUM") as psV, \
             tc.tile_pool(name="psD", bufs=1, space="PSUM") as psD:

            pe_all = pep.tile([P, NT, 1024], BF16, name="pe_all")
            dn_ps = psD.tile([P, NQ], F32, tag="dn", name="dn_ps")

            for c in range(4):
                j0 = 2 * c if causal else 0
                w = (NQ - j0) * P
                for i in range(4):
                    t = 4 * c + i
                    dcol = (t // 2 - j0) * P        # diagonal slot's column
                    for p0 in range(0, w, 512):
                        wp = min(512, w - p0)
                        ps = psS.tile([P, wp], F32, tag="s", name=f"ps{t}_{p0}")
                        for m in range(4):
                            nc.tensor.matmul(
                                ps,
                                lhsT=kt_sb[:, 2 * m:2 * m + 2, t * P:(t + 1) * P],
                                rhs=qt_sb[:, 2 * m:2 * m + 2,
                                          j0 * P + p0:j0 * P + p0 + wp],
                                start=(m == 0), stop=(m == 3),
                                perf_mode=mybir.MatmulPerfMode.DoubleRow)
                        if causal and p0 <= dcol < p0 + wp:
                            nc.vector.tensor_tensor(
                                out=ps[:, dcol - p0:dcol - p0 + P],
                                in0=ps[:, dcol - p0:dcol - p0 + P],
                                in1=msk_sb[:, (t % 2) * P:(t % 2 + 1) * P],
                                op=add)
                        nc.scalar.activation(
                            out=pe_all[:, t, j0 * P + p0:j0 * P + p0 + wp],
                            in_=ps, func=Exp, scale=float(1.0 / np.sqrt(D)))

            def av_slot(j):
                n_t = nkt[j]
                av = psV.tile([P, D], F32, tag="av", name=f"av{j}")
                for t in range(n_t):
                    pblk = pe_all[:, t, j * P:(j + 1) * P]
                    nc.tensor.matmul(
                        dn_ps[:, j:j + 1], lhsT=pblk, rhs=ones_sb,
                        start=(t == 0), stop=(t == n_t - 1))
                    for half in range(2):
                        nc.tensor.matmul(
                            av[:, half * 512:(half + 1) * 512],
                            lhsT=pblk,
                            rhs=v_sb[:, t, half * 512:(half + 1) * 512],
                            start=(t == 0), stop=(t == n_t - 1))
                rec = recp.tile([P, 1], F32, tag="rec", name=f"rec{j}")
                nc.vector.reciprocal(out=rec, in_=dn_ps[:, j:j + 1])
                o = outp.tile([P, D], F32, tag="o", name=f"o{j}")
                last = j == (0 if causal else NQ - 1)
                nhf = 4 if last else 2
                hw = D // nhf
                for hf in range(nhf):
                    nc.vector.scalar_tensor_tensor(
                        out=o[:, hf * hw:(hf + 1) * hw],
                        in0=av[:, hf * hw:(hf + 1) * hw], scalar=rec,
                        in1=bv_sb[:, hf * hw:(hf + 1) * hw],
                        op0=mult, op1=add)
                    eng = nc.sync if hf % 2 == 0 else nc.scalar
                    eng.dma_start(out=out[j, :, hf * hw:(hf + 1) * hw],
                                  in_=o[:, hf * hw:(hf + 1) * hw])

            # ascending so early AVs only need early exps; smallest slot
            # last so the kernel tail is its short drain
            for j in ([1, 2, 3, 4, 5, 6, 7, 0] if causal else list(range(NQ))):
                av_slot(j)

    nc.compile()
    return nc


def _get_program(causal: bool):
    key = bool(causal)
    if key not in _PROG_CACHE:
        _PROG_CACHE[key] = build_program(key)
    return _PROG_CACHE[key]


def _shard_inputs(encoded_q, encoded_k, encoded_v, W_q, b_q, W_k, b_k,
                  W_v, b_v, causal):
    """Build the per-core in_maps (all host-side numpy, bf16 payloads)."""
    F8 = mybir.dt.np(FP8)
    wqh = np.ascontiguousarray(
        (16.0 * W_q.T).reshape(EO, P, D).transpose(1, 0, 2)).astype(F8)
    wkh = np.ascontiguousarray(
        (16.0 * W_k.T).reshape(EO, P, D).transpose(1, 0, 2)).astype(F8)
    wvh = np.ascontiguousarray(
        W_v.T.reshape(EO, P, D).transpose(1, 0, 2)).astype(BF)
    bqh = np.ascontiguousarray(b_q.reshape(DO, P).T)
    bkh = np.ascontiguousarray(b_k.reshape(DO, P).T)
    bvh = np.ascontiguousarray(np.broadcast_to(b_v, (P, D)))

    ki = np.arange(P)[:, None]
    qi = np.arange(P)[None, :]
    tri = np.where(ki <= qi, 0.0, NEG).astype(np.float32)   # diagonal block
    zer = np.zeros((P, P), np.float32)
    ninf = np.full((P, P), NEG, np.float32)
    # h=0: slot j owns g=2j -> k-tile 2j is diagonal, 2j+1 fully masked
    # h=1: slot j owns g=2j+1 -> k-tile 2j unmasked, 2j+1 diagonal
    mskh = [np.concatenate([tri, ninf], 1), np.concatenate([zer, tri], 1)]

    in_maps = []
    for c in range(N_CORES):
        b, h = divmod(c, 2)
        gts = _slot_gtiles(h, causal)
        Xq = np.concatenate([encoded_q[b, g * P:(g + 1) * P, :] for g in gts], 0)
        xqh = np.ascontiguousarray(
            Xq.T.reshape(EO, P, 1024).transpose(1, 0, 2)).astype(F8)
        xkh = np.ascontiguousarray(
            encoded_k[b].T.reshape(EO, P, 4, 512).transpose(2, 1, 0, 3)).astype(F8)
        xvh = np.ascontiguousarray(
            encoded_v[b].T.reshape(EO, P, NT, P).transpose(2, 1, 0, 3)).astype(BF)
        in_maps.append({
            "xq": xqh, "xk": xkh, "xv": xvh,
            "wq": wqh, "wk": wkh, "wv": wvh,
            "bq": bqh, "bk": bkh, "bv": bvh,
            "msk": mskh[h] if causal else np.zeros((P, 2 * P), np.float32),
        })
    return in_maps


def kernel(encoded_q, encoded_k, encoded_v, W_q, b_q, W_k, b_k, W_v, b_v,
           parameter_mask, _want_trace=False, _trace_dir=None):
    causal = bool(np.asarray(parameter_mask).item())
    encoded_q = np.asarray(encoded_q, np.float32)
    encoded_k = np.asarray(encoded_k, np.float32)
    encoded_v = np.asarray(encoded_v, np.float32)
    nc = _get_program(causal)
    in_maps = _shard_inputs(encoded_q, encoded_k, encoded_v,
                            np.asarray(W_q, np.float32), np.asarray(b_q, np.float32),
                            np.asarray(W_k, np.float32), np.asarray(b_k, np.float32),
                            np.asarray(W_v, np.float32), np.asarray(b_v, np.float32),
                            causal)
    kw = {}
    if _want_trace:
        kw = dict(trace=True, tmpdir=_trace_dir)
    res = run_bass_kernel_spmd(nc, in_maps, core_ids=list(range(N_CORES)), **kw)

    full = np.empty((B, S, D), np.float32)
    for c in range(N_CORES):
        b, h = divmod(c, 2)
        o = res.results[c]["out"]
        for j, g in enumerate(_slot_gtiles(h, causal)):
            full[b, g * P:(g + 1) * P, :] = o[j]
    if _want_trace:
        return full, res
    return full
